# revision 8
# baseline (speedup 1.0000x reference)
"""Trainium2 Bass kernel for nn_DenseAttentionMultiHead (dense_transformer).

Reference computation (B=4, S=2048, H=2048, 16 heads, D=128, no softmax):
    x   = rope(hidden_states)                       # [B, S, H]
    q   = x @ W.T
    out = ((q_h @ k_h^T) @ k_h  per head)           # k == roped x heads

Key algebraic rewrite (valid because there is no softmax):
    (Q K^T) K == Q (K^T K)
so the [S, S] intermediate collapses to a [128, 128] Gram matrix per head.
Per-core work is dominated by the q-projection (2048x2048x1024 MACs).

Sharding (8 cores): core c -> (batch b = c // 2, head-group g = c % 2).
Outputs are disjoint -> no collectives. The host rolls the contraction (H)
axis of x^T and W_g^T so each core's own heads sit in rows 0:1024 (tiles
0-7); one SPMD program serves both head-groups.

v2 schedule (from the v1 perfetto profile: PE busy only 63%, input DMA
streamed 26 MB f32, ACT spent ~50us on rope half-swap copies):
  - all inputs host-cast to bf16 (halves HBM traffic, no cast in the DMA)
  - rope half-swap on ACT via f32-bitcast views (halves element count;
    ACT is ~1 elem/cycle dtype-independent)
  - rope products split: DVE does pterm mul + final add, GpSimd does the
    sterm mul (GpSimd has no PSUM port, so it never drains PSUM)
  - phase 1 (input streaming): k-outer q^T accumulation for m0-6 x s-chunk
    0 (N=512 -- the ISA caps matmul output at 512 elements per
    instruction), plus the 8 per-head Gram matrices interleaved on the
    8th PSUM bank (DMA-XBAR transposes feed them)
  - PE warm-up matmuls at t=0 release the HAM clock gate (cold PE runs at
    1.2 GHz for ~3.4us) before real accumulation lands
  - phase 2: per-m k-inner N=512 groups; PSUM->SBUF bf16 drains alternate
    ACT/DVE; out^T_h = G_h q^T_h; out(m) is emitted one group late so its
    qt drain is already complete (no PE stall); output is bf16 (host
    upcasts)
"""

from contextlib import ExitStack

import numpy as np
import ml_dtypes

import concourse.bass as bass
import concourse.tile as tile
from concourse import bacc, mybir
from concourse.bass import ts
from concourse.bass_utils import run_bass_kernel_spmd

B, S, H = 4, 2048, 2048
NH, D = 16, 128
HPC = 8  # heads per core
GCOLS = HPC * D  # 1024 q-columns per core
NKT = H // 128  # 16 partition tiles along H
NMT = GCOLS // 128  # 8 q-column tiles
F32 = mybir.dt.float32
BF16 = mybir.dt.bfloat16


def build_kernel(ctx: ExitStack, tc: tile.TileContext, xt, wt, ccT, ssT, outT):
    nc = tc.nc

    p_cs = ctx.enter_context(tc.tile_pool(name="cs", bufs=1))
    p_xb = ctx.enter_context(tc.tile_pool(name="xb", bufs=3))
    p_sw = ctx.enter_context(tc.tile_pool(name="swp", bufs=2))
    p_pt = ctx.enter_context(tc.tile_pool(name="ptp", bufs=2))
    p_xtr = ctx.enter_context(tc.tile_pool(name="xtr", bufs=NKT))
    p_w = ctx.enter_context(tc.tile_pool(name="wpool", bufs=NKT))
    p_qt = ctx.enter_context(tc.tile_pool(name="qtpool", bufs=NMT))
    p_k = ctx.enter_context(tc.tile_pool(name="kpool", bufs=2))
    p_ms = ctx.enter_context(tc.tile_pool(name="mspool", bufs=NMT))
    p_ost = ctx.enter_context(tc.tile_pool(name="ost", bufs=3))
    ps1 = ctx.enter_context(tc.tile_pool(name="ps1", bufs=7, space="PSUM"))
    ps2 = ctx.enter_context(tc.tile_pool(name="ps2", bufs=1, space="PSUM"))

    # rope tables (host-precomputed, bf16): cc = [cos;cos], ss = [-sin;sin]
    cc = p_cs.tile([128, S], BF16, tag="cc")
    nc.sync.dma_start(out=cc[:], in_=ccT[:, :])
    ss = p_cs.tile([128, S], BF16, tag="ss")
    nc.sync.dma_start(out=ss[:], in_=ssT[:, :])

    # PE warm-up: junk matmuls on the cc table release the HAM clock gate
    # (~3.4us of sustained activity) before the first real work arrives.
    wu = ps2.tile([128, 512], F32, tag="p2", name="warm")
    for i in range(8):
        nc.tensor.matmul(wu[:], cc[:, ts(i, 128)], cc[:, 0:512], start=True, stop=True)

    # phase-1 PSUM accumulators: m0-6 x s-chunk 0 (one bank each); the
    # 8th bank holds the warm-up tile then the Gram accumulator.
    qA = [ps1.tile([128, 512], F32, tag="p1", name=f"qA{m}") for m in range(7)]
    mg = ps2.tile([128, 512], F32, tag="p2", name="gram")

    # prefetch the first two (x, W) tile pairs
    xbq = {}
    for kt in range(2):
        xbq[kt] = p_xb.tile([128, S], BF16, tag="xb", name=f"xb{kt}")
        nc.gpsimd.dma_start(out=xbq[kt][:], in_=xt[ts(kt, 128), :])
    wb = []
    for kt in range(2):
        wtile = p_w.tile([128, GCOLS], BF16, tag="wb", name=f"wb{kt}")
        nc.sync.dma_start(out=wtile[:], in_=wt[ts(kt, 128), :])
        wb.append(wtile)

    # --- phase 1: stream bf16 loads, rope, partial q^T, per-head Gram ---
    xtr = []
    ktn = {}
    msl = {}
    for kt in range(NKT):
        # prefetch tile kt+2 (issues go out ahead of this iteration's
        # compute so the DMA engines stay saturated)
        if kt + 2 < NKT:
            kn = kt + 2
            xbq[kn] = p_xb.tile([128, S], BF16, tag="xb", name=f"xb{kn}")
            nc.gpsimd.dma_start(out=xbq[kn][:], in_=xt[ts(kn, 128), :])
            wtile = p_w.tile([128, GCOLS], BF16, tag="wb", name=f"wb{kn}")
            nc.sync.dma_start(out=wtile[:], in_=wt[ts(kn, 128), :])
            wb.append(wtile)

        xb = xbq.pop(kt)
        # half-swap via ACT cross-partition copies on f32 views (halves
        # the per-copy element count; ACT rate is dtype-independent)
        sw = p_sw.tile([128, S], BF16, tag="sw", name=f"sw{kt}")
        nc.scalar.copy(sw[0:64, :].bitcast(F32), xb[64:128, :].bitcast(F32))
        nc.scalar.copy(sw[64:128, :].bitcast(F32), xb[0:64, :].bitcast(F32))
        pterm = p_pt.tile([128, S], BF16, tag="pterm", name=f"pt{kt}")
        nc.vector.tensor_mul(pterm[:], xb[:], cc[:])
        nc.gpsimd.tensor_mul(sw[:], sw[:], ss[:])
        xr = p_xtr.tile([128, S], BF16, tag="xtr", name=f"xtr{kt}")
        nc.vector.tensor_add(xr[:], pterm[:], sw[:])
        xtr.append(xr)

        # k-outer partial q^T accumulation over the first s-chunk
        for m in range(7):
            nc.tensor.matmul(
                qA[m][:],
                wb[kt][:, ts(m, 128)],
                xr[:, 0:512],
                start=(kt == 0),
                stop=(kt == NKT - 1),
            )

        # own-head Gram pipeline: transpose head kt-1, Gram for head kt-2
        if 1 <= kt <= 8:
            h = kt - 1
            kta = p_k.tile([128, NKT, 128], BF16, tag="ktn", name=f"ktn{h}")
            nc.sync.dma_start_transpose(kta[:], xtr[h][:])
            ktn[h] = kta
        if 2 <= kt <= 9:
            h = kt - 2
            gacc = mg[:, 0:128]
            for cq in range(NKT):
                nc.tensor.matmul(
                    gacc,
                    ktn[h][:, cq, :],
                    ktn[h][:, cq, :],
                    start=(cq == 0),
                    stop=(cq == NKT - 1),
                )
            ms = p_ms.tile([128, 128], BF16, tag="ms", name=f"ms{h}")
            nc.scalar.copy(ms[:], gacc)
            msl[h] = ms

    qt = [p_qt.tile([128, S], BF16, tag="qt", name=f"qt{m}") for m in range(NMT)]

    # PSUM->SBUF drains alternate ACT / DVE (GpSimd has no PSUM port)
    rr = [0]

    def drain(dst_ap, src_ap):
        if rr[0] % 2 == 0:
            nc.scalar.copy(dst_ap, src_ap)
        else:
            nc.vector.tensor_copy(dst_ap, src_ap)
        rr[0] += 1

    # phase-1 boundary: drain the seven accumulators
    for m in range(7):
        drain(qt[m][:, 0:512], qA[m][:])

    # --- phase 2: remaining q^T chunks (k-inner), out matmuls, output ---
    def emit_out(m):
        """out^T_h = G_h @ q^T_h ; four N=512 matmuls, bf16 DMA per chunk."""
        for sc in range(4):
            ops = ps1.tile([128, 512], F32, tag="p1", name=f"o{m}_{sc}")
            nc.tensor.matmul(
                ops[:], msl[m][:], qt[m][:, ts(sc, 512)], start=True, stop=True
            )
            ot = p_ost.tile([128, 512], BF16, tag="ost", name=f"ot{m}_{sc}")
            drain(ot[:], ops[:])
            nc.sync.dma_start(out=outT[ts(m, 128), ts(sc, 512)], in_=ot[:])

    # (m, s-chunk) accumulation groups still owed after phase 1
    owed = [(7, 0)]
    for m in range(NMT):
        for sc in range(1, 4):
            owed.append((m, sc))

    done = {m: (1 if m < 7 else 0) for m in range(NMT)}
    pending = None
    for m, sc in owed:
        acc = ps1.tile([128, 512], F32, tag="p1", name=f"q{m}_{sc}")
        for kt in range(NKT):
            nc.tensor.matmul(
                acc[:],
                wb[kt][:, ts(m, 128)],
                xtr[kt][:, ts(sc, 512)],
                start=(kt == 0),
                stop=(kt == NKT - 1),
            )
        drain(qt[m][:, ts(sc, 512)], acc[:])
        # emit the previous m's out now: its qt drain has had a full
        # group's worth of PE time to complete -> no stall
        if pending is not None:
            emit_out(pending)
            pending = None
        done[m] += 1
        if done[m] == 4:
            pending = m
    emit_out(pending)


_NC_CACHE = {}


def build_nc():
    if "nc" in _NC_CACHE:
        return _NC_CACHE["nc"]
    nc = bacc.Bacc("TRN2", target_bir_lowering=False, debug=False)
    xt = nc.dram_tensor("xt", [H, S], BF16, kind="ExternalInput").ap()
    wt = nc.dram_tensor("wt", [H, GCOLS], BF16, kind="ExternalInput").ap()
    ccT = nc.dram_tensor("cc", [128, S], BF16, kind="ExternalInput").ap()
    ssT = nc.dram_tensor("ss", [128, S], BF16, kind="ExternalInput").ap()
    outT = nc.dram_tensor("outT", [GCOLS, S], BF16, kind="ExternalOutput").ap()
    with tile.TileContext(nc) as tc:
        with ExitStack() as ctx:
            build_kernel(ctx, tc, xt, wt, ccT, ssT, outT)
    nc.compile()
    _NC_CACHE["nc"] = nc
    return nc


def make_in_maps(hidden_states, W, cos, sin):
    """bf16-cast and roll the contraction axis so own heads sit in rows 0:1024."""
    bf16 = ml_dtypes.bfloat16
    hs = np.asarray(hidden_states, dtype=np.float32)
    W = np.asarray(W, dtype=np.float32)
    cosT = np.asarray(cos, dtype=np.float32).T  # [64, S]
    sinT = np.asarray(sin, dtype=np.float32).T
    cc = np.ascontiguousarray(np.concatenate([cosT, cosT], axis=0)).astype(bf16)
    ss = np.ascontiguousarray(np.concatenate([-sinT, sinT], axis=0)).astype(bf16)

    xtb = {b: np.ascontiguousarray(hs[b].T).astype(bf16) for b in range(B)}
    wtb = {
        g: np.ascontiguousarray(W[GCOLS * g : GCOLS * (g + 1), :].T).astype(bf16)
        for g in range(2)
    }
    in_maps = []
    for c in range(8):
        b, g = c // 2, c % 2
        xm = xtb[b]
        wm = wtb[g]
        if g == 1:  # own heads (rows 1024:2048) -> rows 0:1024
            xm = np.roll(xm, -GCOLS, axis=0)
            wm = np.roll(wm, -GCOLS, axis=0)
        in_maps.append(
            {
                "xt": np.ascontiguousarray(xm),
                "wt": np.ascontiguousarray(wm),
                "cc": cc,
                "ss": ss,
            }
        )
    return in_maps


def run(hidden_states, W, cos, sin, trace=False):
    nc = build_nc()
    in_maps = make_in_maps(hidden_states, W, cos, sin)
    res = run_bass_kernel_spmd(nc, in_maps, list(range(8)), trace=trace)
    out = np.empty((B, S, H), np.float32)
    for c in range(8):
        b, g = c // 2, c % 2
        out[b][:, GCOLS * g : GCOLS * (g + 1)] = (
            res.results[c]["outT"].T.astype(np.float32)
        )
    return out, res


def kernel(hidden_states, W, cos, sin):
    out, _ = run(hidden_states, W, cos, sin, trace=False)
    return out


# revision 15
# speedup vs baseline: 1.0865x; 1.0865x over previous
"""Trainium2 Bass kernel for nn_DenseAttentionMultiHead (dense_transformer).

Reference computation (B=4, S=2048, H=2048, 16 heads, D=128, no softmax):
    x   = rope(hidden_states)                       # [B, S, H]
    q   = x @ W.T
    out = ((q_h @ k_h^T) @ k_h  per head)           # k == roped x heads

Key algebraic rewrite (valid because there is no softmax):
    (Q K^T) K == Q (K^T K)
so the [S, S] intermediate collapses to a [128, 128] Gram matrix per head.
Per-core work is dominated by the q-projection (2048x2048x1024 MACs).

Sharding (8 cores): core c -> (batch b = c // 2, head-group g = c % 2).
Outputs are disjoint -> no collectives. The host rolls the contraction (H)
axis of x^T and W_g^T so each core's own heads sit in rows 0:1024 (tiles
0-7); one SPMD program serves both head-groups.

v2 schedule (from the v1 perfetto profile: PE busy only 63%, input DMA
streamed 26 MB f32, ACT spent ~50us on rope half-swap copies):
  - all inputs host-cast to bf16 (halves HBM traffic, no cast in the DMA)
  - rope half-swap on ACT via f32-bitcast views (halves element count;
    ACT is ~1 elem/cycle dtype-independent)
  - rope products split by measured rates (DVE 2x-mode ~0.6 ns/col,
    GpSimd ~1.8 ns/col): GpSimd does ONE large-range sterm mul (cols
    608:2048), DVE does pterm mul + the sterm head + the final add; both
    stages land at ~3.0 us/tile so the pipeline is balanced
  - phase 1 (input streaming): k-outer q^T accumulation for m0-6 x s-chunk
    0 (N=512 -- the ISA caps matmul output at 512 elements per
    instruction), plus the 8 per-head Gram matrices interleaved on the
    8th PSUM bank (DMA-XBAR transposes feed them)
  - PE warm-up matmuls at t=0 release the HAM clock gate (cold PE runs at
    1.2 GHz for ~3.4us) before real accumulation lands
  - phase 2: per-m k-inner N=512 groups; PSUM->SBUF bf16 drains alternate
    ACT/DVE; out^T_h = G_h q^T_h; out(m) is emitted one group late so its
    qt drain is already complete (no PE stall); output is bf16 (host
    upcasts)
"""

from contextlib import ExitStack

import numpy as np
import ml_dtypes

import concourse.bass as bass
import concourse.tile as tile
from concourse import bacc, mybir
from concourse.bass import ts
from concourse.bass_utils import run_bass_kernel_spmd

B, S, H = 4, 2048, 2048
NH, D = 16, 128
HPC = 8  # heads per core
GCOLS = HPC * D  # 1024 q-columns per core
NKT = H // 128  # 16 partition tiles along H
NMT = GCOLS // 128  # 8 q-column tiles
F32 = mybir.dt.float32
BF16 = mybir.dt.bfloat16


def build_kernel(ctx: ExitStack, tc: tile.TileContext, xt, wt, ccT, ssT, outT):
    nc = tc.nc

    p_cs = ctx.enter_context(tc.tile_pool(name="cs", bufs=1))
    p_xb = ctx.enter_context(tc.tile_pool(name="xb", bufs=3))
    p_sw = ctx.enter_context(tc.tile_pool(name="swp", bufs=2))
    p_pt = ctx.enter_context(tc.tile_pool(name="ptp", bufs=2))
    p_xtr = ctx.enter_context(tc.tile_pool(name="xtr", bufs=NKT))
    p_w = ctx.enter_context(tc.tile_pool(name="wpool", bufs=4))
    p_qt = ctx.enter_context(tc.tile_pool(name="qtpool", bufs=NMT))
    p_k = ctx.enter_context(tc.tile_pool(name="kpool", bufs=2))
    p_ms = ctx.enter_context(tc.tile_pool(name="mspool", bufs=NMT))
    p_ost = ctx.enter_context(tc.tile_pool(name="ost", bufs=3))
    ps1 = ctx.enter_context(tc.tile_pool(name="ps1", bufs=7, space="PSUM"))
    ps2 = ctx.enter_context(tc.tile_pool(name="ps2", bufs=1, space="PSUM"))

    # rope tables (host-precomputed, bf16): cc = [cos;cos], ss = [-sin;sin]
    cc = p_cs.tile([128, S], BF16, tag="cc")
    nc.sync.dma_start(out=cc[:], in_=ccT[:, :])
    ss = p_cs.tile([128, S], BF16, tag="ss")
    nc.sync.dma_start(out=ss[:], in_=ssT[:, :])

    # PE warm-up: junk matmuls on the cc table release the HAM clock gate
    # (~3.4us of sustained activity) before the first real work arrives.
    wu = ps2.tile([128, 512], F32, tag="p2", name="warm")
    for i in range(8):
        nc.tensor.matmul(wu[:], cc[:, ts(i, 128)], cc[:, 0:512], start=True, stop=True)

    # phase-1 PSUM accumulators: m0-6 x s-chunk 0 (one bank each); the
    # 8th bank holds the warm-up tile then the Gram accumulator.
    qA = [ps1.tile([128, 512], F32, tag="p1", name=f"qA{m}") for m in range(7)]
    mg = ps2.tile([128, 512], F32, tag="p2", name="gram")

    # x loads in 8 two-tile batches and W loads in 4 four-tile batches --
    # all on the sync queue (fewer, bigger DMA issues).
    def issue_x(pi):
        xq = p_xb.tile([128, 2, S], BF16, tag="xb", name=f"xq{pi}")
        nc.sync.dma_start(
            out=xq[:],
            in_=xt[pi * 256 : (pi + 1) * 256, :].rearrange("(j p) c -> p j c", p=128),
        )
        return xq

    def issue_w(bi):
        wq = p_w.tile([128, 4, GCOLS], BF16, tag="wb", name=f"wq{bi}")
        nc.sync.dma_start(
            out=wq[:],
            in_=wt[bi * 512 : (bi + 1) * 512, :].rearrange("(j p) c -> p j c", p=128),
        )
        return wq

    xqs = {0: issue_x(0)}
    wqs = {0: issue_w(0)}
    xqs[1] = issue_x(1)
    wb = []

    # --- phase 1: stream bf16 loads, rope, partial q^T, per-head Gram ---
    # rope engine split (measured: DVE 2x-mode ~0.6 ns/col, GpSimd ~1.8
    # ns/col): GpSimd handles the sterm product for cols CSPL:, DVE does
    # the rest; both stages come out ~3 us/tile.
    CSPL = 608
    xtr = []
    ktn = {}
    msl = {}
    mg2 = None
    for kt in range(NKT):
        pi = kt // 2
        if kt % 2 == 0 and pi + 2 < 8:
            xqs[pi + 2] = issue_x(pi + 2)
        if kt % 4 == 0 and kt // 4 + 1 < 4:
            wqs[kt // 4 + 1] = issue_w(kt // 4 + 1)
        if kt % 4 == 0:
            for j in range(4):
                wb.append(wqs[kt // 4][:, j, :])

        xb = xqs[pi][:, kt % 2, :]
        # half-swap via ACT cross-partition copies on f32 views (halves
        # the per-copy element count; ACT rate is dtype-independent)
        sw = p_sw.tile([128, S], BF16, tag="sw", name=f"sw{kt}")
        nc.scalar.copy(sw[0:64, :].bitcast(F32), xb[64:128, :].bitcast(F32))
        nc.scalar.copy(sw[64:128, :].bitcast(F32), xb[0:64, :].bitcast(F32))
        pterm = p_pt.tile([128, S], BF16, tag="pterm", name=f"pt{kt}")
        nc.gpsimd.tensor_mul(sw[:, CSPL:S], sw[:, CSPL:S], ss[:, CSPL:S])
        nc.vector.tensor_mul(pterm[:], xb[:], cc[:])
        nc.vector.tensor_mul(sw[:, 0:CSPL], sw[:, 0:CSPL], ss[:, 0:CSPL])
        xr = p_xtr.tile([128, S], BF16, tag="xtr", name=f"xtr{kt}")
        nc.vector.tensor_add(xr[:], pterm[:], sw[:])
        xtr.append(xr)

        # k-outer partial q^T accumulation over the first s-chunk
        for m in range(7):
            nc.tensor.matmul(
                qA[m][:],
                wb[kt][:, ts(m, 128)],
                xr[:, 0:512],
                start=(kt == 0),
                stop=(kt == NKT - 1),
            )
        # m7 s-chunk 0 rides the Gram bank once Gram is done (k-outer
        # catch-up at kt=12, then arrival-paced)
        if kt == 12:
            mg2 = ps2.tile([128, 512], F32, tag="p2", name="qA7")
            for ktp in range(13):
                nc.tensor.matmul(
                    mg2[:],
                    wb[ktp][:, ts(7, 128)],
                    xtr[ktp][:, 0:512],
                    start=(ktp == 0),
                    stop=False,
                )
        elif kt > 12:
            nc.tensor.matmul(
                mg2[:],
                wb[kt][:, ts(7, 128)],
                xtr[kt][:, 0:512],
                start=False,
                stop=(kt == NKT - 1),
            )

        # own-head Gram pipeline: transpose head kt-1, Gram for head kt-2
        if 1 <= kt <= 8:
            h = kt - 1
            kta = p_k.tile([128, NKT, 128], BF16, tag="ktn", name=f"ktn{h}")
            nc.sync.dma_start_transpose(kta[:], xtr[h][:])
            ktn[h] = kta
        if 2 <= kt <= 9:
            h = kt - 2
            gacc = mg[:, 0:128]
            for cq in range(NKT):
                nc.tensor.matmul(
                    gacc,
                    ktn[h][:, cq, :],
                    ktn[h][:, cq, :],
                    start=(cq == 0),
                    stop=(cq == NKT - 1),
                )
            ms = p_ms.tile([128, 128], BF16, tag="ms", name=f"ms{h}")
            nc.scalar.copy(ms[:], gacc)
            msl[h] = ms

    qt = [p_qt.tile([128, S], BF16, tag="qt", name=f"qt{m}") for m in range(NMT)]

    # PSUM->SBUF drains alternate ACT / DVE (GpSimd has no PSUM port)
    rr = [0]

    def drain(dst_ap, src_ap):
        if rr[0] % 2 == 0:
            nc.scalar.copy(dst_ap, src_ap)
        else:
            nc.vector.tensor_copy(dst_ap, src_ap)
        rr[0] += 1

    # phase-1 boundary: drain the eight s-chunk-0 accumulators
    for m in range(7):
        drain(qt[m][:, 0:512], qA[m][:])
    drain(qt[7][:, 0:512], mg2[:])

    # --- phase 2: remaining q^T chunks (k-inner), out matmuls, output ---
    def emit_out(m):
        """out^T_h = G_h @ q^T_h ; four N=512 matmuls, bf16 DMA per chunk."""
        for sc in range(4):
            ops = ps1.tile([128, 512], F32, tag="p1", name=f"o{m}_{sc}")
            nc.tensor.matmul(
                ops[:], msl[m][:], qt[m][:, ts(sc, 512)], start=True, stop=True
            )
            ot = p_ost.tile([128, 512], BF16, tag="ost", name=f"ot{m}_{sc}")
            drain(ot[:], ops[:])
            nc.sync.dma_start(out=outT[ts(m, 128), ts(sc, 512)], in_=ot[:])

    # (m, s-chunk) accumulation groups still owed after phase 1
    owed = []
    for m in range(NMT):
        for sc in range(1, 4):
            owed.append((m, sc))

    done = {m: 1 for m in range(NMT)}
    pending = None
    for m, sc in owed:
        acc = ps1.tile([128, 512], F32, tag="p1", name=f"q{m}_{sc}")
        for kt in range(NKT):
            nc.tensor.matmul(
                acc[:],
                wb[kt][:, ts(m, 128)],
                xtr[kt][:, ts(sc, 512)],
                start=(kt == 0),
                stop=(kt == NKT - 1),
            )
        drain(qt[m][:, ts(sc, 512)], acc[:])
        # emit the previous m's out now: its qt drain has had a full
        # group's worth of PE time to complete -> no stall
        if pending is not None:
            emit_out(pending)
            pending = None
        done[m] += 1
        if done[m] == 4:
            pending = m
    emit_out(pending)


_NC_CACHE = {}


def build_nc():
    if "nc" in _NC_CACHE:
        return _NC_CACHE["nc"]
    nc = bacc.Bacc("TRN2", target_bir_lowering=False, debug=False)
    xt = nc.dram_tensor("xt", [H, S], BF16, kind="ExternalInput").ap()
    wt = nc.dram_tensor("wt", [H, GCOLS], BF16, kind="ExternalInput").ap()
    ccT = nc.dram_tensor("cc", [128, S], BF16, kind="ExternalInput").ap()
    ssT = nc.dram_tensor("ss", [128, S], BF16, kind="ExternalInput").ap()
    outT = nc.dram_tensor("outT", [GCOLS, S], BF16, kind="ExternalOutput").ap()
    with tile.TileContext(nc) as tc:
        with ExitStack() as ctx:
            build_kernel(ctx, tc, xt, wt, ccT, ssT, outT)
    nc.compile()
    _NC_CACHE["nc"] = nc
    return nc


def make_in_maps(hidden_states, W, cos, sin):
    """bf16-cast and roll the contraction axis so own heads sit in rows 0:1024."""
    bf16 = ml_dtypes.bfloat16
    hs = np.asarray(hidden_states, dtype=np.float32)
    W = np.asarray(W, dtype=np.float32)
    cosT = np.asarray(cos, dtype=np.float32).T  # [64, S]
    sinT = np.asarray(sin, dtype=np.float32).T
    cc = np.ascontiguousarray(np.concatenate([cosT, cosT], axis=0)).astype(bf16)
    ss = np.ascontiguousarray(np.concatenate([-sinT, sinT], axis=0)).astype(bf16)

    xtb = {b: np.ascontiguousarray(hs[b].T).astype(bf16) for b in range(B)}
    wtb = {
        g: np.ascontiguousarray(W[GCOLS * g : GCOLS * (g + 1), :].T).astype(bf16)
        for g in range(2)
    }
    in_maps = []
    for c in range(8):
        b, g = c // 2, c % 2
        xm = xtb[b]
        wm = wtb[g]
        if g == 1:  # own heads (rows 1024:2048) -> rows 0:1024
            xm = np.roll(xm, -GCOLS, axis=0)
            wm = np.roll(wm, -GCOLS, axis=0)
        in_maps.append(
            {
                "xt": np.ascontiguousarray(xm),
                "wt": np.ascontiguousarray(wm),
                "cc": cc,
                "ss": ss,
            }
        )
    return in_maps


def run(hidden_states, W, cos, sin, trace=False):
    nc = build_nc()
    in_maps = make_in_maps(hidden_states, W, cos, sin)
    res = run_bass_kernel_spmd(nc, in_maps, list(range(8)), trace=trace)
    out = np.empty((B, S, H), np.float32)
    for c in range(8):
        b, g = c // 2, c % 2
        out[b][:, GCOLS * g : GCOLS * (g + 1)] = (
            res.results[c]["outT"].T.astype(np.float32)
        )
    return out, res


def kernel(hidden_states, W, cos, sin):
    out, _ = run(hidden_states, W, cos, sin, trace=False)
    return out
